# revision 2
# baseline (speedup 1.0000x reference)
"""Gaussian label-splat density kernel for Trainium2 (8 NeuronCores).

Math (matches the reference): for each batch b
    gx[n, w] = exp(-(w - lx[n])^2 / (2 sigma^2))   (normalized over w)
    gy[n, h] = exp(-(h - ly[n])^2 / (2 sigma^2))   (normalized over h)
    density[b, 0] = sum_n outer(gy[n], gx[n]) = gy.T @ gx    (K = 64 labels)

batch_images contributes only its shape, so the kernel never touches it.

Sharding: core c -> (batch b = c // 2, row half t = c % 2, h0 = 256 * t).
Each core builds its own gaussians from a 4 KB label packet and emits a
(256, 512) output tile. No cross-core comms.

Both normalizers are computed analytically (no full-range row-sum on the
critical path): sum_{j in Z} exp(-(j-c)^2/(2 s^2)) = s*sqrt(2 pi) exactly
enough for s >= 1 (Poisson summation; theta correction < 3e-9), so
Z = s*sqrt(2 pi) - left tail - right tail, with each 64-term tail an
explicit exp over a (64, 64) block.  The product 1/(Zx*Zy) folds into the
small y-slice (the matmul lhsT), the rhs is the raw x profile.

Matmul operands are bf16 (PSUM accumulates f32; rel tolerance is 2e-2),
and the x profile exp is split in halves so the first pair of matmuls
starts one ACT op earlier.  The output path is pipelined in four
(128, 256) chunks: PSUM->SBUF copies alternate between Vector and Scalar,
and the four output DMAs alternate between the two HWDGE rings (Sync and
Scalar queues) so each chunk streams out as soon as it is copied.

An input-independent warm-up exp pulls the ~1.5us ACT table load into the
label-DMA wait window; the iota base grid is built there too.

Label packet (built on host), partitions 0..63 = labels, (64, 16) f32:
    col 0 = -lx              (bias for the x square)
    col 1 = M = -1/(2 s^2)   (exp scale)
    col 2 = s*sqrt(2 pi)     (infinite-range gaussian sum)
    col 3 = h0 - ly          (y row-window offset)
    col 4 = lx + 1           (x left tail offset)
    col 5 = 512 - lx         (x right tail offset)
    col 6 = ly + 1           (y left tail offset)
    col 7 = 512 - ly         (y right tail offset)
"""

import numpy as np

import concourse.bacc as bacc
import concourse.tile as tile
from concourse.tile import add_dep_helper
from concourse import mybir
from concourse.bass_utils import run_bass_kernel_spmd

B, NLAB, H, W = 4, 64, 512, 512
P = 128
HALF = H // 2  # output rows per core
NTAIL = 64  # terms per truncation tail
N_CORES = 8
F32 = mybir.dt.float32
BF16 = mybir.dt.bfloat16
SQRT_2PI = 2.5066282746310002

_CACHE: list = []


def _build():
    AF = mybir.ActivationFunctionType
    AX = mybir.AxisListType
    OP = mybir.AluOpType
    nc = bacc.Bacc(
        "TRN2",
        debug=False,
        target_bir_lowering=False,
        num_devices=N_CORES,
        enable_partition_id=False,
    )
    labels = nc.dram_tensor("labels", (NLAB, 16), F32, kind="ExternalInput").ap()
    out = nc.dram_tensor("out", (HALF, W), F32, kind="ExternalOutput").ap()

    with tile.TileContext(nc) as tc:
        with (
            tc.tile_pool(name="sb", bufs=1) as pool,
            tc.tile_pool(name="ob", bufs=2) as opool,
            tc.tile_pool(name="ps", bufs=2, space="PSUM") as psum,
        ):
            # input-independent warm-up op so walrus's ACT_TABLE_LOAD lands
            # here and hides under the label DMA's completion latency
            warm = pool.tile([NLAB, 1], F32)
            nc.vector.memset(warm, 0.0)
            nc.scalar.activation(warm, warm, AF.Exp, scale=1.0)

            L = pool.tile([NLAB, 16], F32)
            nc.sync.dma_start(out=L, in_=labels)

            I = pool.tile([NLAB, W], F32)
            nc.gpsimd.iota(
                I,
                pattern=[[1, W]],
                base=0,
                channel_multiplier=0,
                allow_small_or_imprecise_dtypes=True,
            )

            # ---- tails: cols 0:64 x-left, 64:128 x-right, 128:192 y-left,
            # 192:256 y-right.  DVE does the x pair, GpSimd the y pair.
            Dt = pool.tile([NLAB, 4 * NTAIL], F32)
            nc.vector.tensor_scalar_add(Dt[:, 0:NTAIL], I[:, 0:NTAIL], L[:, 4:5])
            nc.vector.tensor_scalar_add(
                Dt[:, NTAIL : 2 * NTAIL], I[:, 0:NTAIL], L[:, 5:6]
            )
            nc.gpsimd.tensor_scalar_add(
                Dt[:, 2 * NTAIL : 3 * NTAIL], I[:, 0:NTAIL], L[:, 6:7]
            )
            nc.gpsimd.tensor_scalar_add(
                Dt[:, 3 * NTAIL : 4 * NTAIL], I[:, 0:NTAIL], L[:, 7:8]
            )
            SQt = pool.tile([NLAB, 4 * NTAIL], F32)
            nc.vector.tensor_mul(SQt, Dt, Dt)

            # ---- y slice distances (DVE)
            Ds = pool.tile([NLAB, HALF], F32)
            nc.vector.tensor_scalar_add(Ds, I[:, 0:HALF], L[:, 3:4])
            SQs = pool.tile([NLAB, HALF], F32)
            nc.vector.tensor_mul(SQs, Ds, Ds)

            # ---- ACT queue (pinned order): x square -> tails exp ->
            # slice exp -> x exp halves (bf16 rhs)
            SQx = pool.tile([NLAB, W], F32)
            i_sq = nc.scalar.activation(SQx, I, AF.Square, bias=L[:, 0:1], scale=1.0)
            Gt = pool.tile([NLAB, 4 * NTAIL], F32)
            i_gt = nc.scalar.activation(Gt, SQt, AF.Exp, scale=L[:, 1:2])
            Gs = pool.tile([NLAB, HALF], F32)
            i_gs = nc.scalar.activation(Gs, SQs, AF.Exp, scale=L[:, 1:2])
            Gx = pool.tile([NLAB, W], BF16)
            i_gxa = nc.scalar.activation(
                Gx[:, 0:256], SQx[:, 0:256], AF.Exp, scale=L[:, 1:2]
            )
            i_gxb = nc.scalar.activation(
                Gx[:, 256:512], SQx[:, 256:512], AF.Exp, scale=L[:, 1:2]
            )
            add_dep_helper(i_gt.ins, i_sq.ins, sync=False, reason="ACT order")
            add_dep_helper(i_gs.ins, i_gt.ins, sync=False, reason="ACT order")
            add_dep_helper(i_gxa.ins, i_gs.ins, sync=False, reason="ACT order")
            add_dep_helper(i_gxb.ins, i_gxa.ins, sync=False, reason="ACT order")

            # ---- normalizers on DVE: Tx/Ty tail sums, Z = Zfull - T,
            # Rp = 1/(Zx*Zy), GY = Gs * Rp (bf16 lhsT)
            T2 = pool.tile([NLAB, 2], F32)
            nc.vector.reduce_sum(T2[:, 0:1], Gt[:, 0 : 2 * NTAIL], axis=AX.X)
            nc.vector.reduce_sum(T2[:, 1:2], Gt[:, 2 * NTAIL : 4 * NTAIL], axis=AX.X)
            Z2 = pool.tile([NLAB, 2], F32)
            nc.vector.tensor_scalar(Z2, T2, -1.0, L[:, 2:3], OP.mult, OP.add)
            Zp = pool.tile([NLAB, 1], F32)
            nc.vector.tensor_mul(Zp, Z2[:, 0:1], Z2[:, 1:2])
            Rp = pool.tile([NLAB, 1], F32)
            nc.vector.reciprocal(Rp, Zp)
            GY = pool.tile([NLAB, HALF], BF16)
            nc.vector.tensor_scalar_mul(GY, Gs, Rp)

            # ---- matmuls: 2 row-halves (PSUM banks) x 2 x-halves, ordered
            # so both banks' first halves run on Gx[:, 0:256] while ACT is
            # still producing the second x half
            acc0 = psum.tile([P, W], F32)
            acc1 = psum.tile([P, W], F32)
            nc.tensor.matmul(
                acc0[:, 0:256], GY[:, 0:P], Gx[:, 0:256], start=True, stop=True
            )
            nc.tensor.matmul(
                acc1[:, 0:256], GY[:, P:HALF], Gx[:, 0:256], start=True, stop=True
            )
            nc.tensor.matmul(
                acc0[:, 256:512], GY[:, 0:P], Gx[:, 256:512], start=True, stop=True
            )
            nc.tensor.matmul(
                acc1[:, 256:512], GY[:, P:HALF], Gx[:, 256:512], start=True, stop=True
            )

            # ---- store path: four (128, 256) chunks; copies alternate
            # Vector / Scalar, DMAs alternate the Sync / Scalar HWDGE rings
            O1 = opool.tile([P, W], F32)
            O2 = opool.tile([P, W], F32)
            nc.vector.tensor_copy(O1[:, 0:256], acc0[:, 0:256])
            nc.sync.dma_start(out=out[0:P, 0:256], in_=O1[:, 0:256])
            nc.scalar.copy(O2[:, 0:256], acc1[:, 0:256])
            nc.scalar.dma_start(out=out[P:HALF, 0:256], in_=O2[:, 0:256])
            nc.vector.tensor_copy(O1[:, 256:512], acc0[:, 256:512])
            nc.sync.dma_start(out=out[0:P, 256:512], in_=O1[:, 256:512])
            nc.scalar.copy(O2[:, 256:512], acc1[:, 256:512])
            nc.scalar.dma_start(out=out[P:HALF, 256:512], in_=O2[:, 256:512])

    nc.compile()
    return nc


def _in_maps(batch_labels: np.ndarray, sigma: float) -> list:
    maps = []
    inv = -1.0 / (2.0 * sigma * sigma)
    for c in range(N_CORES):
        b, t = divmod(c, 2)
        h0 = t * HALF
        lx = batch_labels[b, :, 0]
        ly = batch_labels[b, :, 1]
        packed = np.zeros((NLAB, 16), np.float32)
        packed[:, 0] = -lx
        packed[:, 1] = inv
        packed[:, 2] = sigma * SQRT_2PI
        packed[:, 3] = h0 - ly
        packed[:, 4] = lx + 1.0
        packed[:, 5] = float(W) - lx
        packed[:, 6] = ly + 1.0
        packed[:, 7] = float(H) - ly
        maps.append({"labels": packed})
    return maps


def _get_nc():
    if not _CACHE:
        _CACHE.append(_build())
    return _CACHE[0]


def _gather(results) -> np.ndarray:
    density = np.empty((B, 1, H, W), np.float32)
    for c in range(N_CORES):
        b, t = divmod(c, 2)
        density[b, 0, t * HALF : (t + 1) * HALF, :] = results[c]["out"]
    return density


def kernel(batch_images, batch_labels, sigma) -> np.ndarray:
    batch_labels = np.asarray(batch_labels, dtype=np.float32)
    sigma = float(np.asarray(sigma))
    nc = _get_nc()
    res = run_bass_kernel_spmd(
        nc, _in_maps(batch_labels, sigma), core_ids=list(range(N_CORES))
    )
    return _gather(res.results)


# revision 3
# speedup vs baseline: 1.0840x; 1.0840x over previous
"""Gaussian label-splat density kernel for Trainium2 (8 NeuronCores).

Math (matches the reference): for each batch b
    gx[n, w] = exp(-(w - lx[n])^2 / (2 sigma^2))   (normalized over w)
    gy[n, h] = exp(-(h - ly[n])^2 / (2 sigma^2))   (normalized over h)
    density[b, 0] = sum_n outer(gy[n], gx[n]) = gy.T @ gx    (K = 64 labels)

batch_images contributes only its shape, so the kernel never touches it.

Sharding: core c -> (batch b = c // 2, row half t = c % 2, h0 = 256 * t).
Each core builds its own gaussians from a 4 KB label packet and emits a
(256, 512) output tile. No cross-core comms.

Both normalizers are computed analytically (no full-range row-sum on the
critical path): sum_{j in Z} exp(-(j-c)^2/(2 s^2)) = s*sqrt(2 pi) exactly
enough for s >= 1 (Poisson summation; theta correction < 3e-9), so
Z = s*sqrt(2 pi) - left tail - right tail, with each 64-term tail an
explicit exp over a (64, 64) block.  The product 1/(Zx*Zy) folds into the
small y-slice (the matmul lhsT); the rhs is the raw x profile.

Schedule notes (from trace analysis):
  - All four tail distance blocks are built in ONE Vector op via a
    stride-0 broadcast AP (GpSimd tensor ops cost ~1.2us each and also
    slow concurrent DVE ops; everything elementwise stays on Vector).
  - Tail sums Tx/Ty come from one 3-D tensor_reduce (64,2,128)->(64,2).
  - Matmul operands are bf16 (PSUM accumulates f32; tolerance is 2e-2).
    The x profile exp is split in halves so the first pair of matmuls
    starts one ACT op earlier; matmuls go 2 row-banks x 2 x-halves.
  - A dozen input-independent bf16 warm-up matmuls run during the label
    DMA wait so the PE HAM clock-gate is at 8/8 when the real matmuls
    issue (~2x matmul rate).
  - PSUM->SBUF copies alternate Vector / Scalar per (128,256) chunk; the
    two output DMAs ride different HWDGE rings (Sync and Scalar), with
    each DMA issued only when its engine has no further copy work, since
    a DMA instruction occupies the issuing engine's queue for ~650ns.
  - An input-independent warm-up exp pulls the ~1.3us ACT table load
    into the label-DMA wait window.

Label packet (built on host), partitions 0..63 = labels, (64, 16) f32:
    col 0 = -lx              (bias for the x square)
    col 1 = M = -1/(2 s^2)   (exp scale)
    col 2 = s*sqrt(2 pi)     (infinite-range gaussian sum)
    col 3 = h0 - ly          (y row-window offset)
    col 4 = lx + 1           (x left tail offset)
    col 5 = 512 - lx         (x right tail offset)
    col 6 = ly + 1           (y left tail offset)
    col 7 = 512 - ly         (y right tail offset)
"""

import numpy as np

import concourse.bacc as bacc
import concourse.tile as tile
from concourse.tile import add_dep_helper
from concourse import mybir
from concourse.bass_utils import run_bass_kernel_spmd

B, NLAB, H, W = 4, 64, 512, 512
P = 128
HALF = H // 2  # output rows per core
NTAIL = 64  # terms per truncation tail
N_CORES = 8
WARM_MMS = 12  # bf16 N=128 dummy matmuls to lift the PE HAM clock gate
F32 = mybir.dt.float32
BF16 = mybir.dt.bfloat16
SQRT_2PI = 2.5066282746310002

_CACHE: list = []


def _build():
    AF = mybir.ActivationFunctionType
    AX = mybir.AxisListType
    OP = mybir.AluOpType
    nc = bacc.Bacc(
        "TRN2",
        debug=False,
        target_bir_lowering=False,
        num_devices=N_CORES,
        enable_partition_id=False,
    )
    labels = nc.dram_tensor("labels", (NLAB, 16), F32, kind="ExternalInput").ap()
    out = nc.dram_tensor("out", (HALF, W), F32, kind="ExternalOutput").ap()

    with tile.TileContext(nc) as tc:
        with (
            tc.tile_pool(name="sb", bufs=1) as pool,
            tc.tile_pool(name="ob", bufs=2) as opool,
            tc.tile_pool(name="ps", bufs=2, space="PSUM") as psum,
        ):
            # input-independent warm-up op so walrus's ACT_TABLE_LOAD lands
            # here and hides under the label DMA's completion latency
            warm = pool.tile([NLAB, 1], F32)
            nc.vector.memset(warm, 0.0)
            nc.scalar.activation(warm, warm, AF.Exp, scale=1.0)

            L = pool.tile([NLAB, 16], F32)
            nc.sync.dma_start(out=L, in_=labels)

            I = pool.tile([NLAB, W], F32)
            nc.gpsimd.iota(
                I,
                pattern=[[1, W]],
                base=0,
                channel_multiplier=0,
                allow_small_or_imprecise_dtypes=True,
            )

            # PE warm-up: keep the PE array busy through the label wait so
            # HAM un-throttles (4/8 -> 8/8) before the real matmuls
            Wb = pool.tile([NLAB, P], BF16)
            nc.vector.memset(Wb, 0.0)
            scr = psum.tile([P, P], F32)
            for _ in range(WARM_MMS):
                nc.tensor.matmul(scr, Wb, Wb, start=True, stop=True)

            # ---- tail distances, one broadcast Vector op:
            # cols 0:64 x-left, 64:128 x-right, 128:192 y-left, 192:256 y-right
            Dt = pool.tile([NLAB, 4 * NTAIL], F32)
            nc.vector.tensor_tensor(
                out=Dt.rearrange("p (a b) -> p a b", a=4),
                in0=I[:, 0:NTAIL].unsqueeze(1).broadcast_to([NLAB, 4, NTAIL]),
                in1=L[:, 4:8].unsqueeze(2).broadcast_to([NLAB, 4, NTAIL]),
                op=OP.add,
            )
            SQt = pool.tile([NLAB, 4 * NTAIL], F32)
            nc.vector.tensor_mul(SQt, Dt, Dt)

            # ---- y slice distances (DVE)
            Ds = pool.tile([NLAB, HALF], F32)
            nc.vector.tensor_scalar_add(Ds, I[:, 0:HALF], L[:, 3:4])
            SQs = pool.tile([NLAB, HALF], F32)
            nc.vector.tensor_mul(SQs, Ds, Ds)

            # ---- ACT queue (pinned order): x square -> tails exp ->
            # slice exp -> x exp halves (bf16 rhs)
            SQx = pool.tile([NLAB, W], F32)
            i_sq = nc.scalar.activation(SQx, I, AF.Square, bias=L[:, 0:1], scale=1.0)
            Gt = pool.tile([NLAB, 4 * NTAIL], F32)
            i_gt = nc.scalar.activation(Gt, SQt, AF.Exp, scale=L[:, 1:2])
            Gs = pool.tile([NLAB, HALF], F32)
            i_gs = nc.scalar.activation(Gs, SQs, AF.Exp, scale=L[:, 1:2])
            Gx = pool.tile([NLAB, W], BF16)
            i_gxa = nc.scalar.activation(
                Gx[:, 0:256], SQx[:, 0:256], AF.Exp, scale=L[:, 1:2]
            )
            i_gxb = nc.scalar.activation(
                Gx[:, 256:512], SQx[:, 256:512], AF.Exp, scale=L[:, 1:2]
            )
            add_dep_helper(i_gt.ins, i_sq.ins, sync=False, reason="ACT order")
            add_dep_helper(i_gs.ins, i_gt.ins, sync=False, reason="ACT order")
            add_dep_helper(i_gxa.ins, i_gs.ins, sync=False, reason="ACT order")
            add_dep_helper(i_gxb.ins, i_gxa.ins, sync=False, reason="ACT order")

            # ---- normalizers on DVE: one 3-D reduce for (Tx, Ty),
            # Z = Zfull - T, Rp = 1/(Zx*Zy), GY = Gs * Rp (bf16 lhsT)
            T2 = pool.tile([NLAB, 2], F32)
            nc.vector.reduce_sum(
                T2, Gt.rearrange("p (a b) -> p a b", a=2), axis=AX.X
            )
            Z2 = pool.tile([NLAB, 2], F32)
            nc.vector.tensor_scalar(Z2, T2, -1.0, L[:, 2:3], OP.mult, OP.add)
            Zp = pool.tile([NLAB, 1], F32)
            nc.vector.tensor_mul(Zp, Z2[:, 0:1], Z2[:, 1:2])
            Rp = pool.tile([NLAB, 1], F32)
            nc.vector.reciprocal(Rp, Zp)
            GY = pool.tile([NLAB, HALF], BF16)
            nc.vector.tensor_scalar_mul(GY, Gs, Rp)

            # ---- matmuls: 2 row-halves (PSUM banks) x 2 x-halves, ordered
            # so both banks' first halves run on Gx[:, 0:256] while ACT is
            # still producing the second x half
            acc0 = psum.tile([P, W], F32)
            acc1 = psum.tile([P, W], F32)
            nc.tensor.matmul(
                acc0[:, 0:256], GY[:, 0:P], Gx[:, 0:256], start=True, stop=True
            )
            nc.tensor.matmul(
                acc1[:, 0:256], GY[:, P:HALF], Gx[:, 0:256], start=True, stop=True
            )
            nc.tensor.matmul(
                acc0[:, 256:512], GY[:, 0:P], Gx[:, 256:512], start=True, stop=True
            )
            nc.tensor.matmul(
                acc1[:, 256:512], GY[:, P:HALF], Gx[:, 256:512], start=True, stop=True
            )

            # ---- store path: copies alternate Vector / Scalar per
            # (128, 256) chunk; one 256 KB DMA per row-half, d1 on the Sync
            # HWDGE ring, d2 on the Scalar ring (issued after ACT's last
            # copy so the ~650ns DMA instruction never delays a copy)
            O1 = opool.tile([P, W], F32)
            O2 = opool.tile([P, W], F32)
            nc.vector.tensor_copy(O1[:, 0:256], acc0[:, 0:256])
            nc.scalar.copy(O2[:, 0:256], acc1[:, 0:256])
            nc.vector.tensor_copy(O1[:, 256:512], acc0[:, 256:512])
            nc.scalar.copy(O2[:, 256:512], acc1[:, 256:512])
            nc.sync.dma_start(out=out[0:P, :], in_=O1)
            nc.scalar.dma_start(out=out[P:HALF, :], in_=O2)

    nc.compile()
    return nc


def _in_maps(batch_labels: np.ndarray, sigma: float) -> list:
    maps = []
    inv = -1.0 / (2.0 * sigma * sigma)
    for c in range(N_CORES):
        b, t = divmod(c, 2)
        h0 = t * HALF
        lx = batch_labels[b, :, 0]
        ly = batch_labels[b, :, 1]
        packed = np.zeros((NLAB, 16), np.float32)
        packed[:, 0] = -lx
        packed[:, 1] = inv
        packed[:, 2] = sigma * SQRT_2PI
        packed[:, 3] = h0 - ly
        packed[:, 4] = lx + 1.0
        packed[:, 5] = float(W) - lx
        packed[:, 6] = ly + 1.0
        packed[:, 7] = float(H) - ly
        maps.append({"labels": packed})
    return maps


def _get_nc():
    if not _CACHE:
        _CACHE.append(_build())
    return _CACHE[0]


def _gather(results) -> np.ndarray:
    density = np.empty((B, 1, H, W), np.float32)
    for c in range(N_CORES):
        b, t = divmod(c, 2)
        density[b, 0, t * HALF : (t + 1) * HALF, :] = results[c]["out"]
    return density


def kernel(batch_images, batch_labels, sigma) -> np.ndarray:
    batch_labels = np.asarray(batch_labels, dtype=np.float32)
    sigma = float(np.asarray(sigma))
    nc = _get_nc()
    res = run_bass_kernel_spmd(
        nc, _in_maps(batch_labels, sigma), core_ids=list(range(N_CORES))
    )
    return _gather(res.results)
